# revision 62
# baseline (speedup 1.0000x reference)
"""AAGNN attention message-passing kernel for 8 TRN2 NeuronCores.

Math: the reference builds a dense masked attention
    att = rownorm(exp(lrelu(A*zi + diag(zj))) * A);  out = relu(z - att @ z)
Since A is 0/1 with self-loops, row i of att has only two distinct values:
    e_off(i)  = exp(lrelu(zi[i]))            on off-diagonal neighbors
    e_diag(i) = exp(lrelu(zi[i] + zj[i]))    on the diagonal
so with deg(i) = rowsum(A)[i], S = (deg-1)*e_off + e_diag:
    att @ z = (e_off/S) * (A@z) + ((e_diag-e_off)/S) * z
    out = relu(beta*z - alpha*(A@z)),  alpha = e_off/S, beta = 1-(e_diag-e_off)/S
This avoids materializing the 8192x8192 attention matrix: one pass over A
(A@[z|1] giving Az and deg) is the entire memory cost.

Sharding: core c owns output rows [c*1024, (c+1)*1024). A is symmetric by
construction, so the column stripe A[:, c0:c0+1024] equals the row shard
transposed - exactly the lhsT layout the TensorEngine wants. Rows of the
stripe (and of inputs) are rolled by -c0 so that z-slot indices are
core-invariant (slots 0..7 are always the local rows), keeping the SPMD
graph identical across cores. No collectives are needed.
"""

import sys

for _p in ("/opt/trn_rl_repo",):
    if _p not in sys.path:
        sys.path.insert(0, _p)

import ml_dtypes
import numpy as np

N = 8192
IN_F = 128
OUT_F = 64
NCORES = 8
M_LOC = N // NCORES      # 1024 rows per core
NT = N // 128            # 64 contraction tiles
MB = M_LOC // 128        # 8 output row-blocks per core
ZW = OUT_F + 1           # 65: z columns + ones column (for deg)
AUGW = OUT_F + 4         # 68: z cols + ones col + zi + zj + pad (fp32r
                         # matmul needs even free-dim counts)
NEG = 0.01               # LeakyReLU slope
KB_PER_CHUNK = 8         # contraction blocks per A-chunk DMA
NCHUNK = NT // KB_PER_CHUNK   # 8 chunk DMAs of [128, 8192]
A_BUFS = NCHUNK          # whole bf16 A stripe fits in SBUF; never throttle

_CACHE = {}


def _emit(tc, nc, bass, mybir, make_identity,
          ashard, inputst, w, b, bc, a1c, a2c, out_ext):
    f32 = mybir.dt.float32
    Act = mybir.ActivationFunctionType
    Alu = mybir.AluOpType

    consts = tc.alloc_tile_pool(name="consts", bufs=1)
    apool = tc.alloc_tile_pool(name="apool", bufs=A_BUFS)
    ep1 = tc.alloc_tile_pool(name="ep1", bufs=2)
    ep64 = tc.alloc_tile_pool(name="ep64", bufs=2)
    outp = tc.alloc_tile_pool(name="outp", bufs=3)

    # ---- constants / setup ------------------------------------------------
    identity = consts.tile([128, 128], f32)
    make_identity(nc, identity)

    w_sb = consts.tile([OUT_F, IN_F], f32)
    nc.sync.dma_start(out=w_sb, in_=w[:, :])

    aa_sb = consts.tile([OUT_F, 2], f32)  # [a1^T | a2^T]
    nc.sync.dma_start(out=aa_sb[:, 0:1], in_=a1c[:, :])
    nc.sync.dma_start(out=aa_sb[:, 1:2], in_=a2c[:, :])

    b_sb = consts.tile([1, OUT_F], f32)
    nc.sync.dma_start(out=b_sb, in_=b[:, :])
    bc_sb = consts.tile([OUT_F, 1], f32)
    nc.sync.dma_start(out=bc_sb, in_=bc[:, :])

    ones1 = consts.tile([1, 128], f32)
    nc.vector.memset(ones1, 1.0)

    f32r = mybir.dt.float32r
    wt_aug = consts.tile([128, AUGW], f32r)   # [W^T | 0 | w1 | w2]
    bias_bc = consts.tile([128, AUGW], f32)   # [b | 1 | a1.b | a2.b] bcast
    rhs_bias = consts.tile([1, AUGW], f32)
    # z_all feeds the big matmul in bf16 (A is 0/1 so bf16 A is exact; bf16
    # z costs ~2e-3 relative on Az only). The ones column (col 64 of each
    # slot) comes from the zero wt_aug column plus the 1.0 in bias_bc, so
    # one tensor_add per slot writes [z | 1]. Local rows additionally keep
    # an fp32 copy for the epilogue (residual + exponentials stay precise).
    bf16 = mybir.dt.bfloat16
    z_all = consts.tile([128, NT * ZW], bf16)  # slot nb: [z|1]
    z_loc32 = consts.tile([128, MB * OUT_F], f32)
    zi_loc = consts.tile([128, 2 * MB], f32)  # local [zi | zj] per block

    pre_psum = tc.alloc_tile_pool(name="pre_psum", bufs=1, space="PSUM")
    wt_psum = pre_psum.tile([128, OUT_F], f32)
    nc.tensor.transpose(wt_psum, w_sb, identity[:OUT_F, :OUT_F])
    nc.vector.tensor_copy(out=wt_aug[:, 0:OUT_F], in_=wt_psum)

    w12_psum = pre_psum.tile([128, 2], f32)
    nc.tensor.matmul(w12_psum, lhsT=w_sb, rhs=aa_sb, start=True, stop=True)
    zcol = consts.tile([128, 1], f32)
    nc.vector.memset(zcol, 0.0)
    nc.vector.tensor_copy(out=wt_aug[:, OUT_F:ZW], in_=zcol)
    nc.vector.tensor_copy(out=wt_aug[:, ZW:ZW + 2], in_=w12_psum)
    nc.vector.tensor_copy(out=wt_aug[:, ZW + 2:AUGW], in_=zcol)

    nc.vector.tensor_copy(out=rhs_bias[0:1, 0:OUT_F], in_=b_sb)
    ab_psum = pre_psum.tile([1, 2], f32)
    nc.tensor.matmul(ab_psum, lhsT=bc_sb, rhs=aa_sb, start=True, stop=True)
    nc.vector.memset(rhs_bias[0:1, OUT_F:ZW], 1.0)
    nc.vector.tensor_copy(out=rhs_bias[0:1, ZW:ZW + 2], in_=ab_psum)
    nc.vector.memset(rhs_bias[0:1, ZW + 2:AUGW], 0.0)

    bias_psum = pre_psum.tile([128, AUGW], f32)
    nc.tensor.matmul(bias_psum, lhsT=ones1, rhs=rhs_bias, start=True, stop=True)
    nc.vector.tensor_copy(out=bias_bc, in_=bias_psum)

    pre_psum.release()

    # ---- fused z production + message-passing matmul ----------------------
    # z slot kb is produced one iteration ahead of the accumulating matmul
    # that consumes it, so PE never stalls on the DVE bias-add, and the A /
    # inputs chunk DMAs stream from t=0.
    in_all = consts.tile([IN_F, N], f32r)
    HALF = M_LOC // 2  # one PSUM bank of f32 per matmul output
    psum2 = tc.alloc_tile_pool(name="psum2", bufs=1, space="PSUM")
    acc_t = [psum2.tile([ZW, HALF], f32, tag=f"acct{h}", name=f"acct{h}")
             for h in range(2)]
    zpsum = tc.alloc_tile_pool(name="zpsum", bufs=4, space="PSUM")

    ICHUNK = N // NCHUNK  # input columns per in_all chunk DMA
    a_tiles = []
    for ci in range(NCHUNK):
        # alternate the two HWDGE rings (SP / ACT) so descriptor dispatch
        # isn't single-queue limited
        eng = nc.sync if ci % 2 == 0 else nc.scalar
        eng.dma_start(
            out=in_all[:, ci * ICHUNK:(ci + 1) * ICHUNK],
            in_=inputst[:, ci * ICHUNK:(ci + 1) * ICHUNK].bitcast(f32r))
        at = apool.tile([128, KB_PER_CHUNK * M_LOC], bf16, tag="at",
                        name=f"at{ci}")
        eng.dma_start(
            out=at,
            in_=ashard[:, ci * KB_PER_CHUNK * M_LOC:
                       (ci + 1) * KB_PER_CHUNK * M_LOC])
        a_tiles.append(at)

    def z_emit(kb):
        zp = zpsum.tile([128, AUGW], f32, tag="zp", name=f"zp{kb}")
        nc.tensor.matmul(zp, lhsT=in_all[:, kb * 128:(kb + 1) * 128],
                         rhs=wt_aug, start=True, stop=True)
        nc.vector.tensor_add(
            z_all[:, kb * ZW:(kb + 1) * ZW], zp[:, 0:ZW], bias_bc[:, 0:ZW])
        if kb < MB:
            nc.vector.tensor_add(
                z_loc32[:, kb * OUT_F:(kb + 1) * OUT_F], zp[:, 0:OUT_F],
                bias_bc[:, 0:OUT_F])
            nc.vector.tensor_add(
                zi_loc[:, kb * 2:kb * 2 + 2], zp[:, ZW:ZW + 2],
                bias_bc[:, ZW:ZW + 2])

    z_emit(0)
    for kb in range(NT):
        if kb + 1 < NT:
            z_emit(kb + 1)
        at = a_tiles[kb // KB_PER_CHUNK]
        j = kb % KB_PER_CHUNK
        zslot = z_all[:, kb * ZW:(kb + 1) * ZW]
        for h in range(2):
            nc.tensor.matmul(
                acc_t[h], lhsT=zslot,
                rhs=at[:, j * M_LOC + h * HALF:j * M_LOC + (h + 1) * HALF],
                start=(kb == 0), stop=(kb == NT - 1))
    zpsum.release()

    # copy [Az|deg]^T to SBUF and transpose each 128-col block back to
    # node-on-partition layout
    azt_sb = consts.tile([ZW, M_LOC], f32)
    for h in range(2):
        nc.vector.tensor_copy(out=azt_sb[:, h * HALF:(h + 1) * HALF],
                              in_=acc_t[h])
    tpsum = tc.alloc_tile_pool(name="tpsum", bufs=3, space="PSUM")

    def acc_for(mb):
        tp = tpsum.tile([128, ZW], f32, tag="tp", name=f"tp{mb}")
        nc.tensor.transpose(tp, azt_sb[:, mb * 128:(mb + 1) * 128],
                            identity[:ZW, :ZW])
        return tp

    # ---- phase 3: per-node epilogue ---------------------------------------
    for mb in range(MB):
        acc = acc_for(mb)
        az = acc[:, 0:OUT_F]
        deg = acc[:, OUT_F:ZW]
        zi = zi_loc[:, 2 * mb:2 * mb + 1]
        zj = zi_loc[:, 2 * mb + 1:2 * mb + 2]
        zloc = z_loc32[:, mb * OUT_F:(mb + 1) * OUT_F]

        s = ep1.tile([128, 1], f32, tag="s")
        nc.vector.tensor_add(s, zi, zj)
        t01 = ep1.tile([128, 1], f32, tag="t01")
        nc.vector.tensor_scalar_mul(t01, zi, NEG)
        l1 = ep1.tile([128, 1], f32, tag="l1")
        nc.vector.tensor_max(l1, zi, t01)
        t02 = ep1.tile([128, 1], f32, tag="t02")
        nc.vector.tensor_scalar_mul(t02, s, NEG)
        l2 = ep1.tile([128, 1], f32, tag="l2")
        nc.vector.tensor_max(l2, s, t02)
        eoff = ep1.tile([128, 1], f32, tag="eoff")
        nc.scalar.activation(eoff, l1, Act.Exp)
        ediag = ep1.tile([128, 1], f32, tag="ediag")
        nc.scalar.activation(ediag, l2, Act.Exp)
        dm1 = ep1.tile([128, 1], f32, tag="dm1")
        nc.vector.tensor_scalar_add(dm1, deg, -1.0)
        Ssum = ep1.tile([128, 1], f32, tag="Ssum")
        nc.vector.scalar_tensor_tensor(
            Ssum, in0=dm1, scalar=eoff, in1=ediag, op0=Alu.mult, op1=Alu.add)
        rS = ep1.tile([128, 1], f32, tag="rS")
        nc.vector.reciprocal(rS, Ssum)
        alpha = ep1.tile([128, 1], f32, tag="alpha")
        nc.vector.tensor_mul(alpha, eoff, rS)
        gd = ep1.tile([128, 1], f32, tag="gd")
        nc.vector.tensor_sub(gd, ediag, eoff)
        gamma = ep1.tile([128, 1], f32, tag="gamma")
        nc.vector.tensor_mul(gamma, gd, rS)
        beta = ep1.tile([128, 1], f32, tag="beta")
        nc.vector.tensor_scalar(
            out=beta, in0=gamma, scalar1=-1.0, scalar2=1.0,
            op0=Alu.mult, op1=Alu.add)

        t1 = ep64.tile([128, OUT_F], f32, tag="t1")
        nc.vector.tensor_scalar_mul(t1, az, alpha)
        h = ep64.tile([128, OUT_F], f32, tag="h")
        nc.vector.scalar_tensor_tensor(
            h, in0=zloc, scalar=beta, in1=t1, op0=Alu.mult, op1=Alu.subtract)
        osb = outp.tile([128, OUT_F], f32, tag="osb")
        nc.scalar.activation(osb, h, Act.Relu)
        nc.sync.dma_start(out=out_ext[mb * 128:(mb + 1) * 128, :], in_=osb)

    tpsum.release()
    psum2.release()
    for p in (outp, ep64, ep1, apool, consts):
        p.release()


def _build():
    import concourse.bass as bass
    import concourse.mybir as mybir
    import concourse.tile as tile
    from concourse import bacc
    from concourse.masks import make_identity

    f32 = mybir.dt.float32
    nc = bacc.Bacc("TRN2", target_bir_lowering=False, debug=False)
    # ashard packed so SBUF partition p's data is one contiguous run:
    # ashard[p, kb*M_LOC + m] = A_rolled_stripe[kb*128 + p, m].
    # bf16: A is 0/1 so the cast is exact, and it halves the dominant
    # HBM stream.
    ashard = nc.declare_dram_parameter("ashard", [128, NT * M_LOC],
                                       mybir.dt.bfloat16, isOutput=False)
    inputst = nc.declare_dram_parameter("inputst", [IN_F, N], f32, isOutput=False)
    w = nc.declare_dram_parameter("w", [OUT_F, IN_F], f32, isOutput=False)
    b = nc.declare_dram_parameter("b", [1, OUT_F], f32, isOutput=False)
    bc = nc.declare_dram_parameter("bc", [OUT_F, 1], f32, isOutput=False)
    a1c = nc.declare_dram_parameter("a1c", [OUT_F, 1], f32, isOutput=False)
    a2c = nc.declare_dram_parameter("a2c", [OUT_F, 1], f32, isOutput=False)
    out_ext = nc.declare_dram_parameter("out", [M_LOC, OUT_F], f32, isOutput=True)

    with tile.TileContext(nc) as tc:
        _emit(tc, nc, bass, mybir, make_identity,
              ashard, inputst, w, b, bc, a1c, a2c, out_ext)
    nc.compile()
    return nc


def _in_maps(inputs):
    A = np.asarray(inputs["A"], dtype=np.float32)
    X = np.asarray(inputs["inputs"], dtype=np.float32)
    W = np.ascontiguousarray(np.asarray(inputs["W"], dtype=np.float32))
    b = np.ascontiguousarray(
        np.asarray(inputs["b"], dtype=np.float32).reshape(1, OUT_F))
    a1 = np.ascontiguousarray(
        np.asarray(inputs["a1"], dtype=np.float32).reshape(1, OUT_F))
    a2 = np.ascontiguousarray(
        np.asarray(inputs["a2"], dtype=np.float32).reshape(1, OUT_F))
    maps = []
    for c in range(NCORES):
        c0 = c * M_LOC
        stripe = A[:, c0:c0 + M_LOC]
        ash = np.concatenate([stripe[c0:], stripe[:c0]], axis=0)
        ash = ash.reshape(NT, 128, M_LOC).transpose(1, 0, 2).reshape(
            128, NT * M_LOC)
        xr = np.concatenate([X[c0:], X[:c0]], axis=0)
        maps.append({
            "ashard": np.ascontiguousarray(ash).astype(ml_dtypes.bfloat16),
            "inputst": np.ascontiguousarray(xr.T),
            "w": W, "b": b,
            "bc": np.ascontiguousarray(b.reshape(OUT_F, 1)),
            "a1c": np.ascontiguousarray(a1.reshape(OUT_F, 1)),
            "a2c": np.ascontiguousarray(a2.reshape(OUT_F, 1)),
        })
    return maps


def _run(inputs, trace=False):
    from concourse.bass_utils import run_bass_kernel_spmd

    if "nc" not in _CACHE:
        _CACHE["nc"] = _build()
    nc = _CACHE["nc"]
    res = run_bass_kernel_spmd(nc, _in_maps(inputs), list(range(NCORES)),
                               trace=trace)
    out = np.concatenate(
        [res.results[c]["out"] for c in range(NCORES)], axis=0)
    return out, res


def kernel(**inputs):
    out, _ = _run(inputs, trace=False)
    return out


def kernel_traced(**inputs):
    out, res = _run(inputs, trace=True)
    return out, res


# revision 68
# speedup vs baseline: 1.2602x; 1.2602x over previous
"""AAGNN attention message-passing kernel for 8 TRN2 NeuronCores.

Math: the reference builds a dense masked attention
    att = rownorm(exp(lrelu(A*zi + diag(zj))) * A);  out = relu(z - att @ z)
Since A is 0/1 with self-loops, row i of att has only two distinct values:
    e_off(i)  = exp(lrelu(zi[i]))            on off-diagonal neighbors
    e_diag(i) = exp(lrelu(zi[i] + zj[i]))    on the diagonal
so with deg(i) = rowsum(A)[i], S = (deg-1)*e_off + e_diag:
    att @ z = (e_off/S) * (A@z) + ((e_diag-e_off)/S) * z
    out = relu(beta*z - alpha*(A@z)),  alpha = e_off/S, beta = 1-(e_diag-e_off)/S
This avoids materializing the 8192x8192 attention matrix: one pass over A
(A@[z|1] giving Az and deg) is the entire memory cost.

Sharding: core c owns output rows [c*1024, (c+1)*1024). A is symmetric by
construction, so the column stripe A[:, c0:c0+1024] equals the row shard
transposed - exactly the lhsT layout the TensorEngine wants. Rows of the
stripe (and of inputs) are rolled by -c0 so that z-slot indices are
core-invariant (slots 0..7 are always the local rows), keeping the SPMD
graph identical across cores. No collectives are needed.
"""

import sys

for _p in ("/opt/trn_rl_repo",):
    if _p not in sys.path:
        sys.path.insert(0, _p)

import ml_dtypes
import numpy as np

N = 8192
IN_F = 128
OUT_F = 64
NCORES = 8
M_LOC = N // NCORES      # 1024 rows per core
NT = N // 128            # 64 contraction tiles
MB = M_LOC // 128        # 8 output row-blocks per core
ZW = OUT_F + 1           # 65: z columns + ones column (for deg)
AUGW = OUT_F + 4         # 68: z cols + ones col + zi + zj + pad (fp32r
                         # matmul needs even free-dim counts)
NEG = 0.01               # LeakyReLU slope
KB_PER_CHUNK = 8         # contraction blocks per A-chunk DMA
NCHUNK = NT // KB_PER_CHUNK   # 8 chunk DMAs of [128, 8192]
A_BUFS = NCHUNK          # whole bf16 A stripe fits in SBUF; never throttle

_CACHE = {}


def _emit(tc, nc, bass, mybir, make_identity,
          ashard, inputst, w, b, bc, a1c, a2c, out_ext):
    f32 = mybir.dt.float32
    Act = mybir.ActivationFunctionType
    Alu = mybir.AluOpType

    consts = tc.alloc_tile_pool(name="consts", bufs=1)
    apool = tc.alloc_tile_pool(name="apool", bufs=A_BUFS)
    ep1 = tc.alloc_tile_pool(name="ep1", bufs=2)
    ep64 = tc.alloc_tile_pool(name="ep64", bufs=2)
    outp = tc.alloc_tile_pool(name="outp", bufs=3)

    # ---- constants / setup ------------------------------------------------
    identity = consts.tile([128, 128], f32)
    make_identity(nc, identity)

    w_sb = consts.tile([OUT_F, IN_F], f32)
    nc.sync.dma_start(out=w_sb, in_=w[:, :])

    aa_sb = consts.tile([OUT_F, 2], f32)  # [a1^T | a2^T]
    nc.sync.dma_start(out=aa_sb[:, 0:1], in_=a1c[:, :])
    nc.sync.dma_start(out=aa_sb[:, 1:2], in_=a2c[:, :])

    b_sb = consts.tile([1, OUT_F], f32)
    nc.sync.dma_start(out=b_sb, in_=b[:, :])
    bc_sb = consts.tile([OUT_F, 1], f32)
    nc.sync.dma_start(out=bc_sb, in_=bc[:, :])

    ones1 = consts.tile([1, 128], f32)
    nc.vector.memset(ones1, 1.0)

    wt_aug = consts.tile([128, AUGW], mybir.dt.bfloat16)  # [W^T|0|w1|w2]
    bias_bc = consts.tile([128, AUGW], f32)   # [b | 1 | a1.b | a2.b] bcast
    rhs_bias = consts.tile([1, AUGW], f32)
    # z_all feeds the big matmul in bf16 (A is 0/1 so bf16 A is exact; bf16
    # z costs ~2e-3 relative on Az only). The ones column (col 64 of each
    # slot) comes from the zero wt_aug column plus the 1.0 in bias_bc, so
    # one tensor_add per slot writes [z | 1]. Local rows additionally keep
    # an fp32 copy for the epilogue (residual + exponentials stay precise).
    bf16 = mybir.dt.bfloat16
    z_all = consts.tile([128, NT * ZW], bf16)  # slot nb: [z|1]
    z_loc32 = consts.tile([128, MB * OUT_F], f32)
    zi_loc = consts.tile([128, 2 * MB], f32)  # local [zi | zj] per block

    pre_psum = tc.alloc_tile_pool(name="pre_psum", bufs=1, space="PSUM")
    wt_psum = pre_psum.tile([128, OUT_F], f32)
    nc.tensor.transpose(wt_psum, w_sb, identity[:OUT_F, :OUT_F])
    nc.vector.tensor_copy(out=wt_aug[:, 0:OUT_F], in_=wt_psum)

    w12_psum = pre_psum.tile([128, 2], f32)
    nc.tensor.matmul(w12_psum, lhsT=w_sb, rhs=aa_sb, start=True, stop=True)
    zcol = consts.tile([128, 1], f32)
    nc.vector.memset(zcol, 0.0)
    nc.vector.tensor_copy(out=wt_aug[:, OUT_F:ZW], in_=zcol)
    nc.vector.tensor_copy(out=wt_aug[:, ZW:ZW + 2], in_=w12_psum)
    nc.vector.tensor_copy(out=wt_aug[:, ZW + 2:AUGW], in_=zcol)

    nc.vector.tensor_copy(out=rhs_bias[0:1, 0:OUT_F], in_=b_sb)
    ab_psum = pre_psum.tile([1, 2], f32)
    nc.tensor.matmul(ab_psum, lhsT=bc_sb, rhs=aa_sb, start=True, stop=True)
    nc.vector.memset(rhs_bias[0:1, OUT_F:ZW], 1.0)
    nc.vector.tensor_copy(out=rhs_bias[0:1, ZW:ZW + 2], in_=ab_psum)
    nc.vector.memset(rhs_bias[0:1, ZW + 2:AUGW], 0.0)

    bias_psum = pre_psum.tile([128, AUGW], f32)
    nc.tensor.matmul(bias_psum, lhsT=ones1, rhs=rhs_bias, start=True, stop=True)
    nc.vector.tensor_copy(out=bias_bc, in_=bias_psum)

    pre_psum.release()

    # ---- fused z production + message-passing matmul ----------------------
    # z slot kb is produced one iteration ahead of the accumulating matmul
    # that consumes it, so PE never stalls on the DVE bias-add, and the A /
    # inputs chunk DMAs stream from t=0.
    in_all = consts.tile([IN_F, N], bf16)
    HALF = M_LOC // 2  # one PSUM bank of f32 per matmul output
    psum2 = tc.alloc_tile_pool(name="psum2", bufs=1, space="PSUM")
    acc_t = [psum2.tile([ZW, HALF], f32, tag=f"acct{h}", name=f"acct{h}")
             for h in range(2)]
    zpsum = tc.alloc_tile_pool(name="zpsum", bufs=4, space="PSUM")

    ICHUNK = N // NCHUNK  # input columns per in_all chunk DMA
    a_tiles = []
    for ci in range(NCHUNK):
        nc.sync.dma_start(
            out=in_all[:, ci * ICHUNK:(ci + 1) * ICHUNK],
            in_=inputst[:, ci * ICHUNK:(ci + 1) * ICHUNK])
        at = apool.tile([128, KB_PER_CHUNK * M_LOC], bf16, tag="at",
                        name=f"at{ci}")
        nc.sync.dma_start(
            out=at,
            in_=ashard[:, ci * KB_PER_CHUNK * M_LOC:
                       (ci + 1) * KB_PER_CHUNK * M_LOC])
        a_tiles.append(at)

    def z_emit(kb):
        zp = zpsum.tile([128, AUGW], f32, tag="zp", name=f"zp{kb}")
        nc.tensor.matmul(zp, lhsT=in_all[:, kb * 128:(kb + 1) * 128],
                         rhs=wt_aug, start=True, stop=True)
        nc.vector.tensor_add(
            z_all[:, kb * ZW:(kb + 1) * ZW], zp[:, 0:ZW], bias_bc[:, 0:ZW])
        if kb < MB:
            nc.vector.tensor_add(
                z_loc32[:, kb * OUT_F:(kb + 1) * OUT_F], zp[:, 0:OUT_F],
                bias_bc[:, 0:OUT_F])
            nc.vector.tensor_add(
                zi_loc[:, kb * 2:kb * 2 + 2], zp[:, ZW:ZW + 2],
                bias_bc[:, ZW:ZW + 2])

    z_emit(0)
    for kb in range(NT):
        if kb + 1 < NT:
            z_emit(kb + 1)
        at = a_tiles[kb // KB_PER_CHUNK]
        j = kb % KB_PER_CHUNK
        zslot = z_all[:, kb * ZW:(kb + 1) * ZW]
        for h in range(2):
            nc.tensor.matmul(
                acc_t[h], lhsT=zslot,
                rhs=at[:, j * M_LOC + h * HALF:j * M_LOC + (h + 1) * HALF],
                start=(kb == 0), stop=(kb == NT - 1))
    zpsum.release()

    # copy [Az|deg]^T to SBUF and transpose each 128-col block back to
    # node-on-partition layout
    azt_sb = consts.tile([ZW, M_LOC], f32)
    for h in range(2):
        nc.vector.tensor_copy(out=azt_sb[:, h * HALF:(h + 1) * HALF],
                              in_=acc_t[h])
    tpsum = tc.alloc_tile_pool(name="tpsum", bufs=3, space="PSUM")

    def acc_for(mb):
        tp = tpsum.tile([128, ZW], f32, tag="tp", name=f"tp{mb}")
        nc.tensor.transpose(tp, azt_sb[:, mb * 128:(mb + 1) * 128],
                            identity[:ZW, :ZW])
        return tp

    # ---- phase 3: per-node epilogue ---------------------------------------
    for mb in range(MB):
        acc = acc_for(mb)
        az = acc[:, 0:OUT_F]
        deg = acc[:, OUT_F:ZW]
        zi = zi_loc[:, 2 * mb:2 * mb + 1]
        zj = zi_loc[:, 2 * mb + 1:2 * mb + 2]
        zloc = z_loc32[:, mb * OUT_F:(mb + 1) * OUT_F]

        s = ep1.tile([128, 1], f32, tag="s")
        nc.vector.tensor_add(s, zi, zj)
        t01 = ep1.tile([128, 1], f32, tag="t01")
        nc.vector.tensor_scalar_mul(t01, zi, NEG)
        l1 = ep1.tile([128, 1], f32, tag="l1")
        nc.vector.tensor_max(l1, zi, t01)
        t02 = ep1.tile([128, 1], f32, tag="t02")
        nc.vector.tensor_scalar_mul(t02, s, NEG)
        l2 = ep1.tile([128, 1], f32, tag="l2")
        nc.vector.tensor_max(l2, s, t02)
        eoff = ep1.tile([128, 1], f32, tag="eoff")
        nc.scalar.activation(eoff, l1, Act.Exp)
        ediag = ep1.tile([128, 1], f32, tag="ediag")
        nc.scalar.activation(ediag, l2, Act.Exp)
        dm1 = ep1.tile([128, 1], f32, tag="dm1")
        nc.vector.tensor_scalar_add(dm1, deg, -1.0)
        Ssum = ep1.tile([128, 1], f32, tag="Ssum")
        nc.vector.scalar_tensor_tensor(
            Ssum, in0=dm1, scalar=eoff, in1=ediag, op0=Alu.mult, op1=Alu.add)
        rS = ep1.tile([128, 1], f32, tag="rS")
        nc.vector.reciprocal(rS, Ssum)
        alpha = ep1.tile([128, 1], f32, tag="alpha")
        nc.vector.tensor_mul(alpha, eoff, rS)
        gd = ep1.tile([128, 1], f32, tag="gd")
        nc.vector.tensor_sub(gd, ediag, eoff)
        gamma = ep1.tile([128, 1], f32, tag="gamma")
        nc.vector.tensor_mul(gamma, gd, rS)
        beta = ep1.tile([128, 1], f32, tag="beta")
        nc.vector.tensor_scalar(
            out=beta, in0=gamma, scalar1=-1.0, scalar2=1.0,
            op0=Alu.mult, op1=Alu.add)

        t1 = ep64.tile([128, OUT_F], f32, tag="t1")
        nc.vector.tensor_scalar_mul(t1, az, alpha)
        h = ep64.tile([128, OUT_F], f32, tag="h")
        nc.vector.scalar_tensor_tensor(
            h, in0=zloc, scalar=beta, in1=t1, op0=Alu.mult, op1=Alu.subtract)
        osb = outp.tile([128, OUT_F], f32, tag="osb")
        nc.scalar.activation(osb, h, Act.Relu)
        nc.sync.dma_start(out=out_ext[mb * 128:(mb + 1) * 128, :], in_=osb)

    tpsum.release()
    psum2.release()
    for p in (outp, ep64, ep1, apool, consts):
        p.release()


def _build():
    import concourse.bass as bass
    import concourse.mybir as mybir
    import concourse.tile as tile
    from concourse import bacc
    from concourse.masks import make_identity

    f32 = mybir.dt.float32
    nc = bacc.Bacc("TRN2", target_bir_lowering=False, debug=False)
    # ashard packed so SBUF partition p's data is one contiguous run:
    # ashard[p, kb*M_LOC + m] = A_rolled_stripe[kb*128 + p, m].
    # bf16: A is 0/1 so the cast is exact, and it halves the dominant
    # HBM stream.
    ashard = nc.declare_dram_parameter("ashard", [128, NT * M_LOC],
                                       mybir.dt.bfloat16, isOutput=False)
    inputst = nc.declare_dram_parameter("inputst", [IN_F, N],
                                        mybir.dt.bfloat16, isOutput=False)
    w = nc.declare_dram_parameter("w", [OUT_F, IN_F], f32, isOutput=False)
    b = nc.declare_dram_parameter("b", [1, OUT_F], f32, isOutput=False)
    bc = nc.declare_dram_parameter("bc", [OUT_F, 1], f32, isOutput=False)
    a1c = nc.declare_dram_parameter("a1c", [OUT_F, 1], f32, isOutput=False)
    a2c = nc.declare_dram_parameter("a2c", [OUT_F, 1], f32, isOutput=False)
    out_ext = nc.declare_dram_parameter("out", [M_LOC, OUT_F], f32, isOutput=True)

    with tile.TileContext(nc) as tc:
        _emit(tc, nc, bass, mybir, make_identity,
              ashard, inputst, w, b, bc, a1c, a2c, out_ext)
    nc.compile()
    return nc


def _in_maps(inputs):
    A = np.asarray(inputs["A"], dtype=np.float32)
    X = np.asarray(inputs["inputs"], dtype=np.float32)
    W = np.ascontiguousarray(np.asarray(inputs["W"], dtype=np.float32))
    b = np.ascontiguousarray(
        np.asarray(inputs["b"], dtype=np.float32).reshape(1, OUT_F))
    a1 = np.ascontiguousarray(
        np.asarray(inputs["a1"], dtype=np.float32).reshape(1, OUT_F))
    a2 = np.ascontiguousarray(
        np.asarray(inputs["a2"], dtype=np.float32).reshape(1, OUT_F))
    maps = []
    for c in range(NCORES):
        c0 = c * M_LOC
        stripe = A[:, c0:c0 + M_LOC]
        ash = np.concatenate([stripe[c0:], stripe[:c0]], axis=0)
        ash = ash.reshape(NT, 128, M_LOC).transpose(1, 0, 2).reshape(
            128, NT * M_LOC)
        xr = np.concatenate([X[c0:], X[:c0]], axis=0)
        maps.append({
            "ashard": np.ascontiguousarray(ash).astype(ml_dtypes.bfloat16),
            "inputst": np.ascontiguousarray(xr.T).astype(ml_dtypes.bfloat16),
            "w": W, "b": b,
            "bc": np.ascontiguousarray(b.reshape(OUT_F, 1)),
            "a1c": np.ascontiguousarray(a1.reshape(OUT_F, 1)),
            "a2c": np.ascontiguousarray(a2.reshape(OUT_F, 1)),
        })
    return maps


def _run(inputs, trace=False):
    from concourse.bass_utils import run_bass_kernel_spmd

    if "nc" not in _CACHE:
        _CACHE["nc"] = _build()
    nc = _CACHE["nc"]
    res = run_bass_kernel_spmd(nc, _in_maps(inputs), list(range(NCORES)),
                               trace=trace)
    out = np.concatenate(
        [res.results[c]["out"] for c in range(NCORES)], axis=0)
    return out, res


def kernel(**inputs):
    out, _ = _run(inputs, trace=False)
    return out


def kernel_traced(**inputs):
    out, res = _run(inputs, trace=True)
    return out, res


# revision 70
# speedup vs baseline: 1.2996x; 1.0313x over previous
"""AAGNN attention message-passing kernel for 8 TRN2 NeuronCores.

Math: the reference builds a dense masked attention
    att = rownorm(exp(lrelu(A*zi + diag(zj))) * A);  out = relu(z - att @ z)
Since A is 0/1 with self-loops, row i of att has only two distinct values:
    e_off(i)  = exp(lrelu(zi[i]))            on off-diagonal neighbors
    e_diag(i) = exp(lrelu(zi[i] + zj[i]))    on the diagonal
so with deg(i) = rowsum(A)[i], S = (deg-1)*e_off + e_diag:
    att @ z = (e_off/S) * (A@z) + ((e_diag-e_off)/S) * z
    out = relu(beta*z - alpha*(A@z)),  alpha = e_off/S, beta = 1-(e_diag-e_off)/S
This avoids materializing the 8192x8192 attention matrix: one pass over A
(A@[z|1] giving Az and deg) is the entire memory cost.

Sharding: core c owns output rows [c*1024, (c+1)*1024). A is symmetric by
construction, so the column stripe A[:, c0:c0+1024] equals the row shard
transposed - exactly the lhsT layout the TensorEngine wants. Rows of the
stripe (and of inputs) are rolled by -c0 so that z-slot indices are
core-invariant (slots 0..7 are always the local rows), keeping the SPMD
graph identical across cores. No collectives are needed.
"""

import sys

for _p in ("/opt/trn_rl_repo",):
    if _p not in sys.path:
        sys.path.insert(0, _p)

import ml_dtypes
import numpy as np

N = 8192
IN_F = 128
OUT_F = 64
NCORES = 8
M_LOC = N // NCORES      # 1024 rows per core
NT = N // 128            # 64 contraction tiles
MB = M_LOC // 128        # 8 output row-blocks per core
ZW = OUT_F + 1           # 65: z columns + ones column (for deg)
AUGW = OUT_F + 4         # 68: z cols + ones col + zi + zj + pad (fp32r
                         # matmul needs even free-dim counts)
NEG = 0.01               # LeakyReLU slope
KB_PER_CHUNK = 8         # contraction blocks per A-chunk DMA
NCHUNK = NT // KB_PER_CHUNK   # 8 chunk DMAs of [128, 8192]
A_BUFS = NCHUNK          # whole bf16 A stripe fits in SBUF; never throttle

_CACHE = {}


def _emit(tc, nc, bass, mybir, make_identity,
          ashard, inputst, w, b, bc, a1c, a2c, out_ext):
    f32 = mybir.dt.float32
    Act = mybir.ActivationFunctionType
    Alu = mybir.AluOpType

    consts = tc.alloc_tile_pool(name="consts", bufs=1)
    apool = tc.alloc_tile_pool(name="apool", bufs=A_BUFS)
    ep1 = tc.alloc_tile_pool(name="ep1", bufs=2)
    ep64 = tc.alloc_tile_pool(name="ep64", bufs=2)
    outp = tc.alloc_tile_pool(name="outp", bufs=3)

    # ---- constants / setup ------------------------------------------------
    identity = consts.tile([128, 128], f32)
    make_identity(nc, identity)

    w_sb = consts.tile([OUT_F, IN_F], f32)
    nc.sync.dma_start(out=w_sb, in_=w[:, :])

    aa_sb = consts.tile([OUT_F, 2], f32)  # [a1^T | a2^T]
    nc.sync.dma_start(out=aa_sb[:, 0:1], in_=a1c[:, :])
    nc.sync.dma_start(out=aa_sb[:, 1:2], in_=a2c[:, :])

    b_sb = consts.tile([1, OUT_F], f32)
    nc.sync.dma_start(out=b_sb, in_=b[:, :])
    bc_sb = consts.tile([OUT_F, 1], f32)
    nc.sync.dma_start(out=bc_sb, in_=bc[:, :])

    ones1 = consts.tile([1, 128], f32)
    nc.vector.memset(ones1, 1.0)

    wt_aug = consts.tile([128, AUGW], mybir.dt.bfloat16)  # [W^T|0|w1|w2]
    bias_bc = consts.tile([128, AUGW], f32)   # [b | 1 | a1.b | a2.b] bcast
    rhs_bias = consts.tile([1, AUGW], f32)
    # z_all feeds the big matmul in bf16 (A is 0/1 so bf16 A is exact; bf16
    # z costs ~2e-3 relative on Az only). The ones column (col 64 of each
    # slot) comes from the zero wt_aug column plus the 1.0 in bias_bc, so
    # one tensor_add per slot writes [z | 1]. Local rows additionally keep
    # an fp32 copy for the epilogue (residual + exponentials stay precise).
    bf16 = mybir.dt.bfloat16
    z_all = consts.tile([128, NT * ZW], bf16)  # slot nb: [z|1]
    z_loc32 = consts.tile([128, MB * OUT_F], f32)
    zi_loc = consts.tile([128, 2 * MB], f32)  # local [zi | zj] per block

    pre_psum = tc.alloc_tile_pool(name="pre_psum", bufs=1, space="PSUM")
    wt_psum = pre_psum.tile([128, OUT_F], f32)
    nc.tensor.transpose(wt_psum, w_sb, identity[:OUT_F, :OUT_F])
    nc.vector.tensor_copy(out=wt_aug[:, 0:OUT_F], in_=wt_psum)

    w12_psum = pre_psum.tile([128, 2], f32)
    nc.tensor.matmul(w12_psum, lhsT=w_sb, rhs=aa_sb, start=True, stop=True)
    zcol = consts.tile([128, 1], f32)
    nc.vector.memset(zcol, 0.0)
    nc.vector.tensor_copy(out=wt_aug[:, OUT_F:ZW], in_=zcol)
    nc.vector.tensor_copy(out=wt_aug[:, ZW:ZW + 2], in_=w12_psum)
    nc.vector.tensor_copy(out=wt_aug[:, ZW + 2:AUGW], in_=zcol)

    nc.vector.tensor_copy(out=rhs_bias[0:1, 0:OUT_F], in_=b_sb)
    ab_psum = pre_psum.tile([1, 2], f32)
    nc.tensor.matmul(ab_psum, lhsT=bc_sb, rhs=aa_sb, start=True, stop=True)
    nc.vector.memset(rhs_bias[0:1, OUT_F:ZW], 1.0)
    nc.vector.tensor_copy(out=rhs_bias[0:1, ZW:ZW + 2], in_=ab_psum)
    nc.vector.memset(rhs_bias[0:1, ZW + 2:AUGW], 0.0)

    bias_psum = pre_psum.tile([128, AUGW], f32)
    nc.tensor.matmul(bias_psum, lhsT=ones1, rhs=rhs_bias, start=True, stop=True)
    nc.vector.tensor_copy(out=bias_bc, in_=bias_psum)

    pre_psum.release()

    # ---- fused z production + message-passing matmul ----------------------
    # z slot kb is produced one iteration ahead of the accumulating matmul
    # that consumes it, so PE never stalls on the DVE bias-add, and the A /
    # inputs chunk DMAs stream from t=0.
    in_all = consts.tile([IN_F, N], bf16)
    HALF = M_LOC // 2  # one PSUM bank of f32 per matmul output
    psum2 = tc.alloc_tile_pool(name="psum2", bufs=1, space="PSUM")
    acc_t = [psum2.tile([ZW, HALF], f32, tag=f"acct{h}", name=f"acct{h}")
             for h in range(2)]
    zpsum = tc.alloc_tile_pool(name="zpsum", bufs=4, space="PSUM")

    ICHUNK = N // NCHUNK  # input columns per in_all chunk DMA
    a_tiles = []
    for ci in range(NCHUNK):
        nc.sync.dma_start(
            out=in_all[:, ci * ICHUNK:(ci + 1) * ICHUNK],
            in_=inputst[:, ci * ICHUNK:(ci + 1) * ICHUNK])
        at = apool.tile([128, KB_PER_CHUNK * M_LOC], bf16, tag="at",
                        name=f"at{ci}")
        nc.sync.dma_start(
            out=at,
            in_=ashard[:, ci * KB_PER_CHUNK * M_LOC:
                       (ci + 1) * KB_PER_CHUNK * M_LOC])
        a_tiles.append(at)

    def z_emit(kb):
        zp = zpsum.tile([128, AUGW], f32, tag="zp", name=f"zp{kb}")
        nc.tensor.matmul(zp, lhsT=in_all[:, kb * 128:(kb + 1) * 128],
                         rhs=wt_aug, start=True, stop=True)
        nc.vector.tensor_add(
            z_all[:, kb * ZW:(kb + 1) * ZW], zp[:, 0:ZW], bias_bc[:, 0:ZW])
        if kb < MB:
            nc.vector.tensor_add(
                z_loc32[:, kb * OUT_F:(kb + 1) * OUT_F], zp[:, 0:OUT_F],
                bias_bc[:, 0:OUT_F])
            nc.vector.tensor_add(
                zi_loc[:, kb * 2:kb * 2 + 2], zp[:, ZW:ZW + 2],
                bias_bc[:, ZW:ZW + 2])

    z_emit(0)
    for kb in range(NT):
        if kb + 1 < NT:
            z_emit(kb + 1)
        at = a_tiles[kb // KB_PER_CHUNK]
        j = kb % KB_PER_CHUNK
        zslot = z_all[:, kb * ZW:(kb + 1) * ZW]
        for h in range(2):
            nc.tensor.matmul(
                acc_t[h], lhsT=zslot,
                rhs=at[:, j * M_LOC + h * HALF:j * M_LOC + (h + 1) * HALF],
                start=(kb == 0), stop=(kb == NT - 1))
    # ---- batched attention-coefficient math (no deg needed) --------------
    # Runs on DVE/ACT during the matmul stream: one [128, 8] op per step
    # instead of 8 separate [128, 1] chains.
    zis = zi_loc.rearrange("p (m t) -> p m t", t=2)[:, :, 0:1]  # [128, 8, 1]
    zjs = zi_loc.rearrange("p (m t) -> p m t", t=2)[:, :, 1:2]
    s8 = ep1.tile([128, MB], f32, tag="s8")
    nc.vector.tensor_add(s8, zis, zjs)
    t8 = ep1.tile([128, MB], f32, tag="t8")
    nc.vector.tensor_scalar_mul(t8, zis, NEG)
    l8 = ep1.tile([128, MB], f32, tag="l8")
    nc.vector.tensor_max(l8, zis, t8)
    eoff8 = ep1.tile([128, MB], f32, tag="eoff8")
    nc.scalar.activation(eoff8, l8, Act.Exp)
    t8b = ep1.tile([128, MB], f32, tag="t8b")
    nc.vector.tensor_scalar_mul(t8b, s8, NEG)
    l8b = ep1.tile([128, MB], f32, tag="l8b")
    nc.vector.tensor_max(l8b, s8, t8b)
    ediag8 = ep1.tile([128, MB], f32, tag="ediag8")
    nc.scalar.activation(ediag8, l8b, Act.Exp)
    gd8 = ep1.tile([128, MB], f32, tag="gd8")
    nc.vector.tensor_sub(gd8, ediag8, eoff8)

    zpsum.release()

    # copy [Az|deg]^T to SBUF
    azt_sb = consts.tile([ZW, M_LOC], f32)
    for h in range(2):
        nc.vector.tensor_copy(out=azt_sb[:, h * HALF:(h + 1) * HALF],
                              in_=acc_t[h])

    # deg row -> node-on-partition via 8 single-column PE transposes into
    # one PSUM tile, then the normalization math batched on [128, 8]
    tpsum = tc.alloc_tile_pool(name="tpsum", bufs=3, space="PSUM")
    dpsum = tc.alloc_tile_pool(name="dpsum", bufs=1, space="PSUM")
    tp_deg = dpsum.tile([128, MB], f32)
    for mb in range(MB):
        nc.tensor.transpose(tp_deg[:, mb:mb + 1],
                            azt_sb[OUT_F:ZW, mb * 128:(mb + 1) * 128],
                            identity[OUT_F:ZW, OUT_F:ZW])
    dm8 = ep1.tile([128, MB], f32, tag="dm8")
    nc.vector.tensor_scalar_add(dm8, tp_deg, -1.0)
    S8 = ep1.tile([128, MB], f32, tag="S8")
    nc.vector.tensor_tensor(out=S8, in0=dm8, in1=eoff8, op=Alu.mult)
    nc.vector.tensor_add(S8, S8, ediag8)
    rS8 = ep1.tile([128, MB], f32, tag="rS8")
    nc.vector.reciprocal(rS8, S8)
    alpha8 = ep1.tile([128, MB], f32, tag="alpha8")
    nc.vector.tensor_mul(alpha8, eoff8, rS8)
    gamma8 = ep1.tile([128, MB], f32, tag="gamma8")
    nc.vector.tensor_mul(gamma8, gd8, rS8)
    beta8 = ep1.tile([128, MB], f32, tag="beta8")
    nc.vector.tensor_scalar(
        out=beta8, in0=gamma8, scalar1=-1.0, scalar2=1.0,
        op0=Alu.mult, op1=Alu.add)

    # ---- per-block: transpose Az back, combine, relu, store ---------------
    for mb in range(MB):
        tp = tpsum.tile([128, ZW], f32, tag="tp", name=f"tp{mb}")
        nc.tensor.transpose(tp, azt_sb[:, mb * 128:(mb + 1) * 128],
                            identity[:ZW, :ZW])
        zloc = z_loc32[:, mb * OUT_F:(mb + 1) * OUT_F]
        t1 = ep64.tile([128, OUT_F], f32, tag="t1")
        nc.vector.tensor_scalar_mul(t1, tp[:, 0:OUT_F],
                                    alpha8[:, mb:mb + 1])
        h = ep64.tile([128, OUT_F], f32, tag="h")
        nc.vector.scalar_tensor_tensor(
            h, in0=zloc, scalar=beta8[:, mb:mb + 1], in1=t1,
            op0=Alu.mult, op1=Alu.subtract)
        osb = outp.tile([128, OUT_F], f32, tag="osb")
        nc.scalar.activation(osb, h, Act.Relu)
        nc.sync.dma_start(out=out_ext[mb * 128:(mb + 1) * 128, :], in_=osb)

    dpsum.release()
    tpsum.release()
    psum2.release()
    for p in (outp, ep64, ep1, apool, consts):
        p.release()


def _build():
    import concourse.bass as bass
    import concourse.mybir as mybir
    import concourse.tile as tile
    from concourse import bacc
    from concourse.masks import make_identity

    f32 = mybir.dt.float32
    nc = bacc.Bacc("TRN2", target_bir_lowering=False, debug=False)
    # ashard packed so SBUF partition p's data is one contiguous run:
    # ashard[p, kb*M_LOC + m] = A_rolled_stripe[kb*128 + p, m].
    # bf16: A is 0/1 so the cast is exact, and it halves the dominant
    # HBM stream.
    ashard = nc.declare_dram_parameter("ashard", [128, NT * M_LOC],
                                       mybir.dt.bfloat16, isOutput=False)
    inputst = nc.declare_dram_parameter("inputst", [IN_F, N],
                                        mybir.dt.bfloat16, isOutput=False)
    w = nc.declare_dram_parameter("w", [OUT_F, IN_F], f32, isOutput=False)
    b = nc.declare_dram_parameter("b", [1, OUT_F], f32, isOutput=False)
    bc = nc.declare_dram_parameter("bc", [OUT_F, 1], f32, isOutput=False)
    a1c = nc.declare_dram_parameter("a1c", [OUT_F, 1], f32, isOutput=False)
    a2c = nc.declare_dram_parameter("a2c", [OUT_F, 1], f32, isOutput=False)
    out_ext = nc.declare_dram_parameter("out", [M_LOC, OUT_F], f32, isOutput=True)

    with tile.TileContext(nc) as tc:
        _emit(tc, nc, bass, mybir, make_identity,
              ashard, inputst, w, b, bc, a1c, a2c, out_ext)
    nc.compile()
    return nc


def _in_maps(inputs):
    A = np.asarray(inputs["A"], dtype=np.float32)
    X = np.asarray(inputs["inputs"], dtype=np.float32)
    W = np.ascontiguousarray(np.asarray(inputs["W"], dtype=np.float32))
    b = np.ascontiguousarray(
        np.asarray(inputs["b"], dtype=np.float32).reshape(1, OUT_F))
    a1 = np.ascontiguousarray(
        np.asarray(inputs["a1"], dtype=np.float32).reshape(1, OUT_F))
    a2 = np.ascontiguousarray(
        np.asarray(inputs["a2"], dtype=np.float32).reshape(1, OUT_F))
    maps = []
    for c in range(NCORES):
        c0 = c * M_LOC
        stripe = A[:, c0:c0 + M_LOC]
        ash = np.concatenate([stripe[c0:], stripe[:c0]], axis=0)
        ash = ash.reshape(NT, 128, M_LOC).transpose(1, 0, 2).reshape(
            128, NT * M_LOC)
        xr = np.concatenate([X[c0:], X[:c0]], axis=0)
        maps.append({
            "ashard": np.ascontiguousarray(ash).astype(ml_dtypes.bfloat16),
            "inputst": np.ascontiguousarray(xr.T).astype(ml_dtypes.bfloat16),
            "w": W, "b": b,
            "bc": np.ascontiguousarray(b.reshape(OUT_F, 1)),
            "a1c": np.ascontiguousarray(a1.reshape(OUT_F, 1)),
            "a2c": np.ascontiguousarray(a2.reshape(OUT_F, 1)),
        })
    return maps


def _run(inputs, trace=False):
    from concourse.bass_utils import run_bass_kernel_spmd

    if "nc" not in _CACHE:
        _CACHE["nc"] = _build()
    nc = _CACHE["nc"]
    res = run_bass_kernel_spmd(nc, _in_maps(inputs), list(range(NCORES)),
                               trace=trace)
    out = np.concatenate(
        [res.results[c]["out"] for c in range(NCORES)], axis=0)
    return out, res


def kernel(**inputs):
    out, _ = _run(inputs, trace=False)
    return out


def kernel_traced(**inputs):
    out, res = _run(inputs, trace=True)
    return out, res
